# revision 4
# baseline (speedup 1.0000x reference)
"""Trainium2 Bass kernel for nn_GRUCell_21612275433682.

Math (from the reference):
  - h0 = 0, so the W_hh matmul is dead: only b_hh enters the gates.
  - y = x @ W_ih.T            (the single big GEMM, [B*T, I] @ [I, 3H])
  - r = (y_r + b_ih_r + b_hh_r > 0)
  - z = (y_z + b_ih_z + b_hh_z > 0)
  - n = (y_n + b_ih_n + r*b_hh_n > 0)
  - cur = (1-z)*n   in {0,1}
  - LIF over T=4 steps:  mem' = 0.99*mem + cur_t - spk_{t-1};  spk_t = (mem' > 1)
    spk_0 is identically 0 (mem1 = cur0 <= 1).

Strategy: pure data parallel over 8 cores (B sharded 256/core). Per core one
[3H=6144, 1024] x [I=2048] GEMM with W stationary ([I,3H] tiles) and X moving,
computed as bf16 hi/lo 3-pass (Whi@Xhi + Whi@Xlo + Wlo@Xhi) accumulated in
fp32 PSUM for fp32-class accuracy at 3 cycles/row. Gate logic + LIF on
DVE/ACT. Layout is [3h partitions, (t-major t*256+b) free] so biases are
per-partition scalars and the LIF is pure free-dim slicing.
"""

import numpy as np
import ml_dtypes

BF16 = ml_dtypes.bfloat16

# Full problem sizes (hardcoded per contract)
B, I, H, T = 2048, 2048, 2048, 4
NCORES = 8
P = 128

_CACHE = {}

# test-harness knobs (grading path leaves these alone)
TRACE = False
LAST_EXEC_NS = None
LAST_RESULTS = None


def build_nc(KT, GJ, BT):
    """Build the per-core Bass program.

    KT: number of 128-wide K tiles (I = 128*KT)
    GJ: number of 128-row h-tile groups per gate (H = 128*GJ)
    BT: batch rows per timestep per core (TB = 4*BT total moving columns)
    """
    import concourse.mybir as mybir
    import concourse.tile as tile
    from concourse import bacc

    TB = 4 * BT
    NT = TB // 512
    assert NT * 512 == TB

    f32 = mybir.dt.float32
    bf16 = mybir.dt.bfloat16
    A = mybir.AluOpType

    nc = bacc.Bacc("TRN2", target_bir_lowering=False, debug=False,
                   num_devices=NCORES)

    xh_d = nc.dram_tensor("xh", [P, KT, TB], bf16, kind="ExternalInput")
    xl_d = nc.dram_tensor("xl", [P, KT, TB], bf16, kind="ExternalInput")
    wh_d = nc.dram_tensor("wh", [GJ, P, 3, KT, P], bf16, kind="ExternalInput")
    wl_d = nc.dram_tensor("wl", [GJ, P, 3, KT, P], bf16, kind="ExternalInput")
    br_d = nc.dram_tensor("br", [P, GJ], f32, kind="ExternalInput")
    bz_d = nc.dram_tensor("bz", [P, GJ], f32, kind="ExternalInput")
    bin_d = nc.dram_tensor("bin", [P, GJ], f32, kind="ExternalInput")
    bhn_d = nc.dram_tensor("bhn", [P, GJ], f32, kind="ExternalInput")
    out_d = nc.dram_tensor("out", [GJ, P, 3 * BT], f32, kind="ExternalOutput")

    with tile.TileContext(nc) as tc:
        with (
            tc.tile_pool(name="xp", bufs=1) as xp,
            tc.tile_pool(name="wp", bufs=2) as wp,
            tc.tile_pool(name="bp", bufs=1) as bp,
            tc.tile_pool(name="gp", bufs=2) as gp,
            tc.tile_pool(name="lp", bufs=2) as lp,
            tc.tile_pool(name="op", bufs=2) as op,
            tc.tile_pool(name="pp", bufs=7, space="PSUM") as pp,
        ):
            xh_sb = xp.tile([P, KT, TB], bf16, tag="xh")
            nc.sync.dma_start(out=xh_sb[:], in_=xh_d[:])
            xl_sb = xp.tile([P, KT, TB], bf16, tag="xl")
            nc.sync.dma_start(out=xl_sb[:], in_=xl_d[:])
            br_sb = bp.tile([P, GJ], f32, tag="br")
            nc.sync.dma_start(out=br_sb[:], in_=br_d[:])
            bz_sb = bp.tile([P, GJ], f32, tag="bz")
            nc.sync.dma_start(out=bz_sb[:], in_=bz_d[:])
            bin_sb = bp.tile([P, GJ], f32, tag="bin")
            nc.sync.dma_start(out=bin_sb[:], in_=bin_d[:])
            bhn_sb = bp.tile([P, GJ], f32, tag="bhn")
            nc.sync.dma_start(out=bhn_sb[:], in_=bhn_d[:])

            for j in range(GJ):
                wh_sb = wp.tile([P, 3, KT, P], bf16, tag="wh")
                nc.sync.dma_start(out=wh_sb[:], in_=wh_d[j])
                wl_sb = wp.tile([P, 3, KT, P], bf16, tag="wl")
                nc.sync.dma_start(out=wl_sb[:], in_=wl_d[j])

                cur = gp.tile([P, TB], f32, tag="cur")
                for n in range(NT):
                    ns = slice(n * 512, (n + 1) * 512)
                    ps = []
                    for g in range(3):
                        pst = pp.tile([P, 512], f32, tag="ps")
                        for k in range(KT):
                            xh_k = xh_sb[:, k, ns]
                            xl_k = xl_sb[:, k, ns]
                            nc.tensor.matmul(pst[:], wh_sb[:, g, k, :], xh_k,
                                             start=(k == 0), stop=False)
                            nc.tensor.matmul(pst[:], wh_sb[:, g, k, :], xl_k,
                                             start=False, stop=False)
                            nc.tensor.matmul(pst[:], wl_sb[:, g, k, :], xh_k,
                                             start=False, stop=(k == KT - 1))
                        ps.append(pst)

                    bj = lambda t: t[:, j:j + 1]
                    # r = (y_r + b_r) > 0 ; zbar = (y_z + b_z) <= 0
                    r = gp.tile([P, 512], f32, tag="r")
                    nc.vector.tensor_scalar(r[:], ps[0][:], bj(br_sb), 0.0,
                                            A.add, A.is_gt)
                    zb = gp.tile([P, 512], f32, tag="zb")
                    nc.vector.tensor_scalar(zb[:], ps[1][:], bj(bz_sb), 0.0,
                                            A.add, A.is_le)
                    # nn = y_n + b_in   (ACT engine, matches ref rounding)
                    nn = gp.tile([P, 512], f32, tag="nn")
                    nc.scalar.activation(nn[:], ps[2][:],
                                         mybir.ActivationFunctionType.Identity,
                                         bias=bj(bin_sb), scale=1.0)
                    # n2 = r*b_hn + nn ;  cur = (n2 > 0) * zbar
                    n2 = gp.tile([P, 512], f32, tag="n2")
                    nc.vector.scalar_tensor_tensor(n2[:], r[:], bj(bhn_sb),
                                                   nn[:], A.mult, A.add)
                    nc.vector.scalar_tensor_tensor(cur[:, ns], n2[:], 0.0,
                                                   zb[:], A.is_gt, A.mult)

                # LIF over the 4 timesteps (t-major layout in cur)
                out_sb = op.tile([P, 3 * BT], f32, tag="out")
                c0 = cur[:, 0 * BT:1 * BT]
                c1 = cur[:, 1 * BT:2 * BT]
                c2 = cur[:, 2 * BT:3 * BT]
                c3 = cur[:, 3 * BT:4 * BT]
                s1 = out_sb[:, 0 * BT:1 * BT]
                s2 = out_sb[:, 1 * BT:2 * BT]
                s3 = out_sb[:, 2 * BT:3 * BT]

                m2 = lp.tile([P, BT], f32, tag="m2")
                nc.vector.scalar_tensor_tensor(m2[:], c0, 0.99, c1,
                                               A.mult, A.add)
                nc.vector.tensor_scalar(s1, m2[:], 1.0, None, A.is_gt)
                t2 = lp.tile([P, BT], f32, tag="t2")
                nc.vector.tensor_tensor(t2[:], c2, s1, A.subtract)
                m3 = lp.tile([P, BT], f32, tag="m3")
                nc.vector.scalar_tensor_tensor(m3[:], m2[:], 0.99, t2[:],
                                               A.mult, A.add)
                nc.vector.tensor_scalar(s2, m3[:], 1.0, None, A.is_gt)
                t3 = lp.tile([P, BT], f32, tag="t3")
                nc.vector.tensor_tensor(t3[:], c3, s2, A.subtract)
                m4 = lp.tile([P, BT], f32, tag="m4")
                nc.vector.scalar_tensor_tensor(m4[:], m3[:], 0.99, t3[:],
                                               A.mult, A.add)
                nc.vector.tensor_scalar(s3, m4[:], 1.0, None, A.is_gt)

                nc.sync.dma_start(out=out_d[j], in_=out_sb[:])

    nc.compile()
    return nc


def prep_weights(W_ih, b_ih, b_hh, KT, GJ):
    """Host-side packing of weights/biases (shared across cores)."""
    threeH = 3 * GJ * P
    II = KT * P
    Wt = np.ascontiguousarray(W_ih[:threeH, :II].T)          # [I, 3H] fp32
    # blocked layout: (j, p, g, k, m) = Wt[k*P+p, (g*GJ+j)*P+m]
    Wb = Wt.reshape(KT, P, 3, GJ, P).transpose(3, 1, 2, 0, 4)
    Wb = np.ascontiguousarray(Wb)
    wh = Wb.astype(BF16)
    wl = (Wb - wh.astype(np.float32)).astype(BF16)

    HH = GJ * P
    b_r = (b_ih[0:HH] + b_hh[0:HH]).astype(np.float32)
    b_z = (b_ih[HH:2 * HH] + b_hh[HH:2 * HH]).astype(np.float32)
    b_in = b_ih[2 * HH:3 * HH].astype(np.float32)
    b_hn = b_hh[2 * HH:3 * HH].astype(np.float32)
    asb = lambda b: np.ascontiguousarray(b.reshape(GJ, P).T)
    return wh, wl, asb(b_r), asb(b_z), asb(b_in), asb(b_hn)


def prep_x(x_core, KT, BT):
    """x_core: [BL, I, T] fp32 -> xh, xl [P, KT, 4*BT] bf16 (t-major cols)."""
    II = KT * P
    xt = x_core[:, :II, :].transpose(1, 2, 0)        # [I, T, BL]
    xt = xt.reshape(KT, P, 4 * BT).transpose(1, 0, 2)  # [P, KT, TB]
    xt = np.ascontiguousarray(xt)
    xh = xt.astype(BF16)
    xl = (xt - xh.astype(np.float32)).astype(BF16)
    return xh, xl


def unpack_out(out, GJ, BT):
    """out: [GJ, P, 3*BT] fp32 -> spikes [BL, H', 4] with t=0 zeros."""
    HH = GJ * P
    arr = out.reshape(HH, 3, BT)                     # [h, t-1, b]
    res = np.zeros((BT, HH, 4), dtype=np.float32)
    res[:, :, 1:4] = arr.transpose(2, 0, 1)
    return res


def kernel(inputs, W_ih, b_ih, W_hh, b_hh):
    from concourse.bass_utils import run_bass_kernel_spmd

    # BT = batch rows per timestep per core (= local batch size BL)
    KT, GJ, BT = I // P, H // P, B // NCORES
    key = (KT, GJ, BT)
    if key not in _CACHE:
        _CACHE[key] = build_nc(KT, GJ, BT)
    nc = _CACHE[key]

    wh, wl, br, bz, bin_, bhn = prep_weights(
        np.asarray(W_ih, dtype=np.float32),
        np.asarray(b_ih, dtype=np.float32),
        np.asarray(b_hh, dtype=np.float32), KT, GJ)

    x = np.asarray(inputs, dtype=np.float32)
    in_maps = []
    BL = B // NCORES
    for c in range(NCORES):
        xh, xl = prep_x(x[c * BL:(c + 1) * BL], KT, BT)
        in_maps.append({"xh": xh, "xl": xl, "wh": wh, "wl": wl,
                        "br": br, "bz": bz, "bin": bin_, "bhn": bhn})

    res = run_bass_kernel_spmd(nc, in_maps, list(range(NCORES)), trace=TRACE)
    global LAST_EXEC_NS, LAST_RESULTS
    LAST_EXEC_NS = res.exec_time_ns
    LAST_RESULTS = res

    out = np.empty((B, H, T), dtype=np.float32)
    for c in range(NCORES):
        out[c * BL:(c + 1) * BL] = unpack_out(res.results[c]["out"], GJ, BT)
    return out


# revision 5
# speedup vs baseline: 1.4244x; 1.4244x over previous
"""Trainium2 Bass kernel for nn_GRUCell_21612275433682.

Math (from the reference):
  - h0 = 0, so the W_hh matmul is dead: only b_hh enters the gates.
  - y = x @ W_ih.T            (the single big GEMM, [B*T, I] @ [I, 3H])
  - r = (y_r + b_ih_r + b_hh_r > 0)
  - z = (y_z + b_ih_z + b_hh_z > 0)
  - n = (y_n + b_ih_n + r*b_hh_n > 0)
  - cur = (1-z)*n   in {0,1}
  - LIF over T=4 steps:  mem' = 0.99*mem + cur_t - spk_{t-1};  spk_t = (mem' > 1)
    spk_0 is identically 0 (mem1 = cur0 <= 1).

Strategy: pure data parallel over 8 cores (B sharded 256/core). Per core one
[3H=6144, TB=1024] x [I=2048] GEMM with W stationary ([I,3H] tiles) and X
moving. Output layout [3h partitions, (t-major t*BT+b) free] so biases are
per-partition scalars and the LIF is pure free-dim slicing.

Two GEMM precision schemes (SCHEME):
  - "bf16x3":  W,X split into bf16 hi/lo; 3 bf16 passes (hi*hi + hi*lo +
               lo*hi) accumulated in fp32 PSUM.  3 cycles/row.
  - "f16f8":   W,X split into fp16 hi/lo; 1 fp16 pass (hi*hi, products are
               exact in fp32) + both cross terms (hi*lo + lo*hi) packed into
               one fp8e4m3 DoubleRow pass.  Everything is pre-scaled by
               powers of two to a common 2^16 PSUM scale so all passes
               accumulate into one PSUM bank; the gate thresholds absorb the
               scale.  2 cycles/row (fp16 1.0 + fp8-DR ~0.56).
"""

import numpy as np
import ml_dtypes

BF16 = ml_dtypes.bfloat16
FP8 = ml_dtypes.float8_e4m3

# Full problem sizes (hardcoded per contract)
B, I, H, T = 2048, 2048, 2048, 4
NCORES = 8
P = 128

SCHEME = "f16f8"

# scheme f16f8 scale choices (powers of two; see product-scale table below)
#   main:  (wh * 2^8) @ (xh * 2^8)            -> y_main * 2^16
#   cross: fp8(wh*2^5) @ fp8(xl*2^11)         -> cross1 * 2^16
#          fp8(wl*2^16) @ fp8(xh)             -> cross2 * 2^16
SW_H, SX_H = 256.0, 256.0
SW8_H, SX8_L = 32.0, 2048.0
SW8_L, SX8_H = 65536.0, 1.0
SCALE = 65536.0

_CACHE = {}

# test-harness knobs (grading path leaves these alone)
TRACE = False
LAST_EXEC_NS = None
LAST_RESULTS = None


def _common_io(nc, mybir, KT, GJ, TB):
    f32 = mybir.dt.float32
    br_d = nc.dram_tensor("br", [P, GJ], f32, kind="ExternalInput")
    bz_d = nc.dram_tensor("bz", [P, GJ], f32, kind="ExternalInput")
    bin_d = nc.dram_tensor("bin", [P, GJ], f32, kind="ExternalInput")
    bhn_d = nc.dram_tensor("bhn", [P, GJ], f32, kind="ExternalInput")
    out_d = nc.dram_tensor("out", [GJ, P, 3 * (TB // 4)], f32,
                           kind="ExternalOutput")
    return br_d, bz_d, bin_d, bhn_d, out_d


def build_nc(KT, GJ, BT, scheme=None):
    """Build the per-core Bass program.

    KT: number of 128-wide K tiles (I = 128*KT)
    GJ: number of 128-row h-tile groups per gate (H = 128*GJ)
    BT: batch rows per timestep per core (TB = 4*BT total moving columns)
    """
    import concourse.mybir as mybir
    import concourse.tile as tile
    from concourse import bacc

    scheme = scheme or SCHEME
    TB = 4 * BT
    NT = TB // 512
    assert NT * 512 == TB

    f32 = mybir.dt.float32
    bf16 = mybir.dt.bfloat16
    f16 = mybir.dt.float16
    f8 = mybir.dt.float8e4
    A = mybir.AluOpType
    DR = mybir.MatmulPerfMode.DoubleRow

    nc = bacc.Bacc("TRN2", target_bir_lowering=False, debug=False,
                   num_devices=NCORES)

    if scheme == "bf16x3":
        xh_d = nc.dram_tensor("xh", [P, KT, TB], bf16, kind="ExternalInput")
        xl_d = nc.dram_tensor("xl", [P, KT, TB], bf16, kind="ExternalInput")
        wh_d = nc.dram_tensor("wh", [GJ, P, 3, KT, P], bf16,
                              kind="ExternalInput")
        wl_d = nc.dram_tensor("wl", [GJ, P, 3, KT, P], bf16,
                              kind="ExternalInput")
    else:
        xh_d = nc.dram_tensor("xh", [P, KT, TB], f16, kind="ExternalInput")
        x8_d = nc.dram_tensor("x8", [P, KT, 2, TB], f8, kind="ExternalInput")
        wh_d = nc.dram_tensor("wh", [GJ, P, 3, KT, P], f16,
                              kind="ExternalInput")
        w8_d = nc.dram_tensor("w8", [GJ, P, 3, KT, 2, P], f8,
                              kind="ExternalInput")
    br_d, bz_d, bin_d, bhn_d, out_d = _common_io(nc, mybir, KT, GJ, TB)

    with tile.TileContext(nc) as tc:
        with (
            tc.tile_pool(name="xp", bufs=1) as xp,
            tc.tile_pool(name="wp", bufs=2) as wp,
            tc.tile_pool(name="bp", bufs=1) as bp,
            tc.tile_pool(name="gp", bufs=2) as gp,
            tc.tile_pool(name="lp", bufs=2) as lp,
            tc.tile_pool(name="op", bufs=2) as op,
            tc.tile_pool(name="pp", bufs=7, space="PSUM") as pp,
        ):
            if scheme == "bf16x3":
                xh_sb = xp.tile([P, KT, TB], bf16, tag="xh")
                nc.sync.dma_start(out=xh_sb[:], in_=xh_d[:])
                x2_sb = xp.tile([P, KT, TB], bf16, tag="x2")
                nc.sync.dma_start(out=x2_sb[:], in_=xl_d[:])
            else:
                xh_sb = xp.tile([P, KT, TB], f16, tag="xh")
                nc.sync.dma_start(out=xh_sb[:], in_=xh_d[:])
                x2_sb = xp.tile([P, KT, 2, TB], f8, tag="x2")
                nc.sync.dma_start(out=x2_sb[:], in_=x8_d[:])

            br_sb = bp.tile([P, GJ], f32, tag="br")
            nc.sync.dma_start(out=br_sb[:], in_=br_d[:])
            bz_sb = bp.tile([P, GJ], f32, tag="bz")
            nc.sync.dma_start(out=bz_sb[:], in_=bz_d[:])
            bin_sb = bp.tile([P, GJ], f32, tag="bin")
            nc.sync.dma_start(out=bin_sb[:], in_=bin_d[:])
            bhn_sb = bp.tile([P, GJ], f32, tag="bhn")
            nc.sync.dma_start(out=bhn_sb[:], in_=bhn_d[:])

            for j in range(GJ):
                if scheme == "bf16x3":
                    wh_sb = wp.tile([P, 3, KT, P], bf16, tag="wh")
                    nc.sync.dma_start(out=wh_sb[:], in_=wh_d[j])
                    w2_sb = wp.tile([P, 3, KT, P], bf16, tag="w2")
                    nc.sync.dma_start(out=w2_sb[:], in_=wl_d[j])
                else:
                    wh_sb = wp.tile([P, 3, KT, P], f16, tag="wh")
                    nc.sync.dma_start(out=wh_sb[:], in_=wh_d[j])
                    w2_sb = wp.tile([P, 3, KT, 2, P], f8, tag="w2")
                    nc.sync.dma_start(out=w2_sb[:], in_=w8_d[j])

                cur = gp.tile([P, TB], f32, tag="cur")
                for n in range(NT):
                    ns = slice(n * 512, (n + 1) * 512)
                    ps = []
                    for g in range(3):
                        pst = pp.tile([P, 512], f32, tag="ps")
                        if scheme == "bf16x3":
                            for k in range(KT):
                                xh_k = xh_sb[:, k, ns]
                                xl_k = x2_sb[:, k, ns]
                                nc.tensor.matmul(pst[:], wh_sb[:, g, k, :],
                                                 xh_k, start=(k == 0),
                                                 stop=False)
                                nc.tensor.matmul(pst[:], wh_sb[:, g, k, :],
                                                 xl_k, start=False, stop=False)
                                nc.tensor.matmul(pst[:], w2_sb[:, g, k, :],
                                                 xh_k, start=False,
                                                 stop=(k == KT - 1))
                        else:
                            for k in range(KT):
                                nc.tensor.matmul(pst[:], wh_sb[:, g, k, :],
                                                 xh_sb[:, k, ns],
                                                 start=(k == 0), stop=False)
                                nc.tensor.matmul(pst[:],
                                                 w2_sb[:, g, k, :, :],
                                                 x2_sb[:, k, :, ns],
                                                 perf_mode=DR, start=False,
                                                 stop=(k == KT - 1))
                        ps.append(pst)

                    bj = lambda t: t[:, j:j + 1]
                    r = gp.tile([P, 512], f32, tag="r")
                    zb = gp.tile([P, 512], f32, tag="zb")
                    if scheme == "bf16x3":
                        # r = (y_r + b_r) > 0 ; zbar = (y_z + b_z) <= 0
                        nc.vector.tensor_scalar(r[:], ps[0][:], bj(br_sb), 0.0,
                                                A.add, A.is_gt)
                        nc.vector.tensor_scalar(zb[:], ps[1][:], bj(bz_sb),
                                                0.0, A.add, A.is_le)
                        # nn = y_n + b_in (ACT engine, matches ref rounding)
                        nn = gp.tile([P, 512], f32, tag="nn")
                        nc.scalar.activation(
                            nn[:], ps[2][:],
                            mybir.ActivationFunctionType.Identity,
                            bias=bj(bin_sb), scale=1.0)
                        # n2 = r*b_hn + nn ;  cur = (n2 > 0) * zbar
                        n2 = gp.tile([P, 512], f32, tag="n2")
                        nc.vector.scalar_tensor_tensor(n2[:], r[:], bj(bhn_sb),
                                                       nn[:], A.mult, A.add)
                        nc.vector.scalar_tensor_tensor(cur[:, ns], n2[:], 0.0,
                                                       zb[:], A.is_gt, A.mult)
                    else:
                        # psum holds y*2^16; br/bz arrive pre-scaled by
                        # -2^16 so the compare absorbs bias and scale.
                        nc.vector.tensor_scalar(r[:], ps[0][:], bj(br_sb),
                                                None, A.is_gt)
                        nc.vector.tensor_scalar(zb[:], ps[1][:], bj(bz_sb),
                                                None, A.is_le)
                        # rbn = r*b_hn + b_in ; n2 = y_n*2^-16 + rbn
                        rbn = gp.tile([P, 512], f32, tag="rbn")
                        nc.vector.tensor_scalar(rbn[:], r[:], bj(bhn_sb),
                                                bj(bin_sb), A.mult, A.add)
                        n2 = gp.tile([P, 512], f32, tag="n2")
                        nc.vector.scalar_tensor_tensor(n2[:], ps[2][:],
                                                       1.0 / SCALE, rbn[:],
                                                       A.mult, A.add)
                        nc.vector.scalar_tensor_tensor(cur[:, ns], n2[:], 0.0,
                                                       zb[:], A.is_gt, A.mult)

                # LIF over the 4 timesteps (t-major layout in cur)
                out_sb = op.tile([P, 3 * BT], f32, tag="out")
                c0 = cur[:, 0 * BT:1 * BT]
                c1 = cur[:, 1 * BT:2 * BT]
                c2 = cur[:, 2 * BT:3 * BT]
                c3 = cur[:, 3 * BT:4 * BT]
                s1 = out_sb[:, 0 * BT:1 * BT]
                s2 = out_sb[:, 1 * BT:2 * BT]
                s3 = out_sb[:, 2 * BT:3 * BT]

                m2 = lp.tile([P, BT], f32, tag="m2")
                nc.vector.scalar_tensor_tensor(m2[:], c0, 0.99, c1,
                                               A.mult, A.add)
                nc.vector.tensor_scalar(s1, m2[:], 1.0, None, A.is_gt)
                t2 = lp.tile([P, BT], f32, tag="t2")
                nc.vector.tensor_tensor(t2[:], c2, s1, A.subtract)
                m3 = lp.tile([P, BT], f32, tag="m3")
                nc.vector.scalar_tensor_tensor(m3[:], m2[:], 0.99, t2[:],
                                               A.mult, A.add)
                nc.vector.tensor_scalar(s2, m3[:], 1.0, None, A.is_gt)
                t3 = lp.tile([P, BT], f32, tag="t3")
                nc.vector.tensor_tensor(t3[:], c3, s2, A.subtract)
                m4 = lp.tile([P, BT], f32, tag="m4")
                nc.vector.scalar_tensor_tensor(m4[:], m3[:], 0.99, t3[:],
                                               A.mult, A.add)
                nc.vector.tensor_scalar(s3, m4[:], 1.0, None, A.is_gt)

                nc.sync.dma_start(out=out_d[j], in_=out_sb[:])

    nc.compile()
    return nc


def _blocked_w(Wt, KT, GJ):
    """[I, 3H] -> (j, p, g, k, m) blocked layout."""
    Wb = Wt.reshape(KT, P, 3, GJ, P).transpose(3, 1, 2, 0, 4)
    return np.ascontiguousarray(Wb)


def prep_weights(W_ih, b_ih, b_hh, KT, GJ, scheme=None):
    """Host-side packing of weights/biases (shared across cores)."""
    scheme = scheme or SCHEME
    threeH = 3 * GJ * P
    II = KT * P
    Wt = np.ascontiguousarray(W_ih[:threeH, :II].T)          # [I, 3H] fp32

    HH = GJ * P
    b_r = (b_ih[0:HH] + b_hh[0:HH]).astype(np.float32)
    b_z = (b_ih[HH:2 * HH] + b_hh[HH:2 * HH]).astype(np.float32)
    b_in = b_ih[2 * HH:3 * HH].astype(np.float32)
    b_hn = b_hh[2 * HH:3 * HH].astype(np.float32)
    asb = lambda b: np.ascontiguousarray(b.reshape(GJ, P).T)

    if scheme == "bf16x3":
        Wb = _blocked_w(Wt, KT, GJ)
        wh = Wb.astype(BF16)
        wl = (Wb - wh.astype(np.float32)).astype(BF16)
        return {"wh": wh, "wl": wl, "br": asb(b_r), "bz": asb(b_z),
                "bin": asb(b_in), "bhn": asb(b_hn)}

    Wb = _blocked_w(Wt, KT, GJ)                              # [GJ,P,3,KT,P]
    wh16 = Wb.astype(np.float16)
    wl = Wb - wh16.astype(np.float32)
    wh_scaled = (wh16.astype(np.float32) * SW_H).astype(np.float16)
    w8 = np.empty(Wb.shape[:4] + (2, P), dtype=FP8)
    w8[..., 0, :] = (wh16.astype(np.float32) * SW8_H).astype(FP8)
    w8[..., 1, :] = (wl * SW8_L).astype(FP8)
    return {"wh": wh_scaled, "w8": w8,
            "br": asb(-b_r * SCALE), "bz": asb(-b_z * SCALE),
            "bin": asb(b_in), "bhn": asb(b_hn)}


def prep_x(x_core, KT, BT, scheme=None):
    """x_core: [BL, I, T] fp32 -> per-core input dict (t-major cols)."""
    scheme = scheme or SCHEME
    II = KT * P
    xt = x_core[:, :II, :].transpose(1, 2, 0)          # [I, T, BL]
    xt = xt.reshape(KT, P, 4 * BT).transpose(1, 0, 2)  # [P, KT, TB]
    xt = np.ascontiguousarray(xt)
    if scheme == "bf16x3":
        xh = xt.astype(BF16)
        xl = (xt - xh.astype(np.float32)).astype(BF16)
        return {"xh": xh, "xl": xl}
    xh16 = xt.astype(np.float16)
    xl = xt - xh16.astype(np.float32)
    xh_scaled = (xh16.astype(np.float32) * SX_H).astype(np.float16)
    x8 = np.empty((P, KT, 2, 4 * BT), dtype=FP8)
    x8[:, :, 0, :] = (xl * SX8_L).astype(FP8)
    x8[:, :, 1, :] = xh16.astype(np.float32).astype(FP8)
    return {"xh": xh_scaled, "x8": x8}


def unpack_out(out, GJ, BT):
    """out: [GJ, P, 3*BT] fp32 -> spikes [BL, H', 4] with t=0 zeros."""
    HH = GJ * P
    arr = out.reshape(HH, 3, BT)                     # [h, t-1, b]
    res = np.zeros((BT, HH, 4), dtype=np.float32)
    res[:, :, 1:4] = arr.transpose(2, 0, 1)
    return res


def kernel(inputs, W_ih, b_ih, W_hh, b_hh):
    from concourse.bass_utils import run_bass_kernel_spmd

    # BT = batch rows per timestep per core (= local batch size BL)
    KT, GJ, BT = I // P, H // P, B // NCORES
    key = (KT, GJ, BT, SCHEME)
    if key not in _CACHE:
        _CACHE[key] = build_nc(KT, GJ, BT)
    nc = _CACHE[key]

    wmap = prep_weights(np.asarray(W_ih, dtype=np.float32),
                        np.asarray(b_ih, dtype=np.float32),
                        np.asarray(b_hh, dtype=np.float32), KT, GJ)

    x = np.asarray(inputs, dtype=np.float32)
    in_maps = []
    BL = B // NCORES
    for c in range(NCORES):
        m = dict(wmap)
        m.update(prep_x(x[c * BL:(c + 1) * BL], KT, BT))
        in_maps.append(m)

    res = run_bass_kernel_spmd(nc, in_maps, list(range(NCORES)), trace=TRACE)
    global LAST_EXEC_NS, LAST_RESULTS
    LAST_EXEC_NS = res.exec_time_ns
    LAST_RESULTS = res

    out = np.empty((B, H, T), dtype=np.float32)
    for c in range(NCORES):
        out[c * BL:(c + 1) * BL] = unpack_out(res.results[c]["out"], GJ, BT)
    return out


# revision 8
# speedup vs baseline: 1.4386x; 1.0100x over previous
"""Trainium2 Bass kernel for nn_GRUCell_21612275433682.

Math (from the reference):
  - h0 = 0, so the W_hh matmul is dead: only b_hh enters the gates.
  - y = x @ W_ih.T            (the single big GEMM, [B*T, I] @ [I, 3H])
  - r = (y_r + b_ih_r + b_hh_r > 0)
  - z = (y_z + b_ih_z + b_hh_z > 0)
  - n = (y_n + b_ih_n + r*b_hh_n > 0)
  - cur = (1-z)*n   in {0,1}
  - LIF over T=4 steps:  mem' = 0.99*mem + cur_t - spk_{t-1};  spk_t = (mem' > 1)
    spk_0 is identically 0 (mem1 = cur0 <= 1).

Strategy: pure data parallel over 8 cores (B sharded 256/core). Per core one
[3H=6144, TB=1024] x [I=2048] GEMM with W stationary ([I,3H] tiles) and X
moving. Output layout [3h partitions, (t-major t*BT+b) free] so biases are
per-partition scalars and the LIF is pure free-dim slicing.

Two GEMM precision schemes (SCHEME):
  - "bf16x3":  W,X split into bf16 hi/lo; 3 bf16 passes (hi*hi + hi*lo +
               lo*hi) accumulated in fp32 PSUM.  3 cycles/row.
  - "f16f8":   W,X split into fp16 hi/lo; 1 fp16 pass (hi*hi, products are
               exact in fp32) + both cross terms (hi*lo + lo*hi) packed into
               one fp8e4m3 DoubleRow pass.  Everything is pre-scaled by
               powers of two to a common 2^16 PSUM scale so all passes
               accumulate into one PSUM bank; the gate thresholds absorb the
               scale.  2 cycles/row (fp16 1.0 + fp8-DR ~0.56).
"""

import numpy as np
import ml_dtypes

BF16 = ml_dtypes.bfloat16
FP8 = ml_dtypes.float8_e4m3

# Full problem sizes (hardcoded per contract)
B, I, H, T = 2048, 2048, 2048, 4
NCORES = 8
P = 128

SCHEME = "f16f8"

# scheme f16f8 scale choices (powers of two; see product-scale table below)
#   main:  (wh * 2^8) @ (xh * 2^8)            -> y_main * 2^16
#   cross: fp8(wh*2^5) @ fp8(xl*2^11)         -> cross1 * 2^16
#          fp8(wl*2^16) @ fp8(xh)             -> cross2 * 2^16
SW_H, SX_H = 256.0, 256.0
SW8_H, SX8_L = 32.0, 2048.0
SW8_L, SX8_H = 65536.0, 1.0
SCALE = 65536.0

_CACHE = {}

# test-harness knobs (grading path leaves these alone)
TRACE = False
LAST_EXEC_NS = None
LAST_RESULTS = None


def _common_io(nc, mybir, KT, GJ, TB):
    f32 = mybir.dt.float32
    br_d = nc.dram_tensor("br", [P, GJ], f32, kind="ExternalInput")
    bz_d = nc.dram_tensor("bz", [P, GJ], f32, kind="ExternalInput")
    bin_d = nc.dram_tensor("bin", [P, GJ], f32, kind="ExternalInput")
    bhn_d = nc.dram_tensor("bhn", [P, GJ], f32, kind="ExternalInput")
    out_d = nc.dram_tensor("out", [GJ, P, 3 * (TB // 4)], f32,
                           kind="ExternalOutput")
    return br_d, bz_d, bin_d, bhn_d, out_d


def build_nc(KT, GJ, BT, scheme=None):
    """Build the per-core Bass program.

    KT: number of 128-wide K tiles (I = 128*KT)
    GJ: number of 128-row h-tile groups per gate (H = 128*GJ)
    BT: batch rows per timestep per core (TB = 4*BT total moving columns)
    """
    import concourse.mybir as mybir
    import concourse.tile as tile
    from concourse import bacc

    scheme = scheme or SCHEME
    TB = 4 * BT
    NT = TB // 512
    assert NT * 512 == TB

    f32 = mybir.dt.float32
    bf16 = mybir.dt.bfloat16
    f16 = mybir.dt.float16
    f8 = mybir.dt.float8e4
    A = mybir.AluOpType
    DR = mybir.MatmulPerfMode.DoubleRow

    nc = bacc.Bacc("TRN2", target_bir_lowering=False, debug=False,
                   num_devices=NCORES)

    if scheme == "bf16x3":
        xh_d = nc.dram_tensor("xh", [P, KT, TB], bf16, kind="ExternalInput")
        xl_d = nc.dram_tensor("xl", [P, KT, TB], bf16, kind="ExternalInput")
        wh_d = nc.dram_tensor("wh", [GJ, P, 3, KT, P], bf16,
                              kind="ExternalInput")
        wl_d = nc.dram_tensor("wl", [GJ, P, 3, KT, P], bf16,
                              kind="ExternalInput")
    else:
        xh_d = nc.dram_tensor("xh", [P, KT, TB], f16, kind="ExternalInput")
        x8_d = nc.dram_tensor("x8", [P, KT, 2, TB], f8, kind="ExternalInput")
        wh_d = nc.dram_tensor("wh", [GJ, P, 3, KT, P], f16,
                              kind="ExternalInput")
        w8_d = nc.dram_tensor("w8", [GJ, P, 3, KT, 2, P], f8,
                              kind="ExternalInput")
    br_d, bz_d, bin_d, bhn_d, out_d = _common_io(nc, mybir, KT, GJ, TB)

    with tile.TileContext(nc) as tc:
        with (
            tc.tile_pool(name="xp", bufs=1) as xp,
            tc.tile_pool(name="wp", bufs=2) as wp,
            tc.tile_pool(name="bp", bufs=1) as bp,
            tc.tile_pool(name="gp", bufs=2) as gp,
            tc.tile_pool(name="lp", bufs=2) as lp,
            tc.tile_pool(name="op", bufs=2) as op,
            tc.tile_pool(name="pp", bufs=7, space="PSUM") as pp,
        ):
            # X arrives on the ACT HWDGE ring in k-chunks so the first
            # matmuls (and the W loads on the sync ring) aren't stuck
            # behind one monolithic 8MB transfer.
            XC = 4 if KT % 4 == 0 else 1
            if scheme == "bf16x3":
                xh_sb = xp.tile([P, KT, TB], bf16, tag="xh")
                x2_sb = xp.tile([P, KT, TB], bf16, tag="x2")
                for c in range(0, KT, XC):
                    cs = slice(c, c + XC)
                    nc.scalar.dma_start(out=xh_sb[:, cs], in_=xh_d[:, cs])
                    nc.scalar.dma_start(out=x2_sb[:, cs], in_=xl_d[:, cs])
            else:
                xh_sb = xp.tile([P, KT, TB], f16, tag="xh")
                x2_sb = xp.tile([P, KT, 2, TB], f8, tag="x2")
                for c in range(0, KT, XC):
                    cs = slice(c, c + XC)
                    nc.scalar.dma_start(out=xh_sb[:, cs], in_=xh_d[:, cs])
                    nc.scalar.dma_start(out=x2_sb[:, cs], in_=x8_d[:, cs])

            br_sb = bp.tile([P, GJ], f32, tag="br")
            nc.gpsimd.dma_start(out=br_sb[:], in_=br_d[:])
            bz_sb = bp.tile([P, GJ], f32, tag="bz")
            nc.gpsimd.dma_start(out=bz_sb[:], in_=bz_d[:])
            bin_sb = bp.tile([P, GJ], f32, tag="bin")
            nc.gpsimd.dma_start(out=bin_sb[:], in_=bin_d[:])
            bhn_sb = bp.tile([P, GJ], f32, tag="bhn")
            nc.gpsimd.dma_start(out=bhn_sb[:], in_=bhn_d[:])

            for j in range(GJ):
                if scheme == "bf16x3":
                    wh_sb = wp.tile([P, 3, KT, P], bf16, tag="wh")
                    nc.sync.dma_start(out=wh_sb[:], in_=wh_d[j])
                    w2_sb = wp.tile([P, 3, KT, P], bf16, tag="w2")
                    nc.sync.dma_start(out=w2_sb[:], in_=wl_d[j])
                else:
                    wh_sb = wp.tile([P, 3, KT, P], f16, tag="wh")
                    nc.sync.dma_start(out=wh_sb[:], in_=wh_d[j])
                    w2_sb = wp.tile([P, 3, KT, 2, P], f8, tag="w2")
                    nc.sync.dma_start(out=w2_sb[:], in_=w8_d[j])

                cur = gp.tile([P, TB], f32, tag="cur")
                nsl = [slice(n * 512, (n + 1) * 512) for n in range(NT)]
                if scheme == "bf16x3":
                    psg = [[pp.tile([P, 512], f32, tag="ps",
                                     name=f"ps_{j}_{g}_{n}")
                            for n in range(NT)] for g in range(3)]
                    for g in range(3):
                        for n in range(NT):
                            pst = psg[g][n]
                            ns = nsl[n]
                            for k in range(KT):
                                xh_k = xh_sb[:, k, ns]
                                xl_k = x2_sb[:, k, ns]
                                nc.tensor.matmul(pst[:], wh_sb[:, g, k, :],
                                                 xh_k, start=(k == 0),
                                                 stop=False)
                                nc.tensor.matmul(pst[:], wh_sb[:, g, k, :],
                                                 xl_k, start=False, stop=False)
                                nc.tensor.matmul(pst[:], w2_sb[:, g, k, :],
                                                 xh_k, start=False,
                                                 stop=(k == KT - 1))
                else:
                    # Alternate 213ns fp16 MMs with 120ns fp8-DR MMs across
                    # all 3*NT PSUM banks so every 256-col DR weight-load
                    # hides under a preceding fp16 MM.
                    psg = [[pp.tile([P, 512], f32, tag="ps",
                                     name=f"ps_{j}_{g}_{n}")
                            for n in range(NT)] for g in range(3)]
                    for k in range(KT):
                        for g in range(3):
                            w16 = wh_sb[:, g, k, :]
                            w8k = w2_sb[:, g, k, :, :]
                            for n in range(NT):
                                nc.tensor.matmul(psg[g][n][:], w16,
                                                 xh_sb[:, k, nsl[n]],
                                                 start=(k == 0), stop=False,
                                                 skip_group_check=True)
                                nc.tensor.matmul(psg[g][n][:], w8k,
                                                 x2_sb[:, k, :, nsl[n]],
                                                 perf_mode=DR, start=False,
                                                 stop=(k == KT - 1),
                                                 skip_group_check=True)

                for n in range(NT):
                    ns = nsl[n]
                    ps = [psg[0][n], psg[1][n], psg[2][n]]
                    bj = lambda t: t[:, j:j + 1]
                    r = gp.tile([P, 512], f32, tag="r")
                    zb = gp.tile([P, 512], f32, tag="zb")
                    if scheme == "bf16x3":
                        # r = (y_r + b_r) > 0 ; zbar = (y_z + b_z) <= 0
                        nc.vector.tensor_scalar(r[:], ps[0][:], bj(br_sb), 0.0,
                                                A.add, A.is_gt)
                        nc.vector.tensor_scalar(zb[:], ps[1][:], bj(bz_sb),
                                                0.0, A.add, A.is_le)
                        # nn = y_n + b_in (ACT engine, matches ref rounding)
                        nn = gp.tile([P, 512], f32, tag="nn")
                        nc.scalar.activation(
                            nn[:], ps[2][:],
                            mybir.ActivationFunctionType.Identity,
                            bias=bj(bin_sb), scale=1.0)
                        # n2 = r*b_hn + nn ;  cur = (n2 > 0) * zbar
                        n2 = gp.tile([P, 512], f32, tag="n2")
                        nc.vector.scalar_tensor_tensor(n2[:], r[:], bj(bhn_sb),
                                                       nn[:], A.mult, A.add)
                        nc.vector.scalar_tensor_tensor(cur[:, ns], n2[:], 0.0,
                                                       zb[:], A.is_gt, A.mult)
                    else:
                        # psum holds y*2^16; br/bz arrive pre-scaled by
                        # -2^16 so the compare absorbs bias and scale.
                        nc.vector.tensor_scalar(r[:], ps[0][:], bj(br_sb),
                                                None, A.is_gt)
                        nc.vector.tensor_scalar(zb[:], ps[1][:], bj(bz_sb),
                                                None, A.is_le)
                        # rbn = r*b_hn + b_in ; n2 = y_n*2^-16 + rbn
                        rbn = gp.tile([P, 512], f32, tag="rbn")
                        nc.vector.tensor_scalar(rbn[:], r[:], bj(bhn_sb),
                                                bj(bin_sb), A.mult, A.add)
                        n2 = gp.tile([P, 512], f32, tag="n2")
                        nc.vector.scalar_tensor_tensor(n2[:], ps[2][:],
                                                       1.0 / SCALE, rbn[:],
                                                       A.mult, A.add)
                        nc.vector.scalar_tensor_tensor(cur[:, ns], n2[:], 0.0,
                                                       zb[:], A.is_gt, A.mult)

                # LIF over the 4 timesteps (t-major layout in cur)
                out_sb = op.tile([P, 3 * BT], f32, tag="out")
                c0 = cur[:, 0 * BT:1 * BT]
                c1 = cur[:, 1 * BT:2 * BT]
                c2 = cur[:, 2 * BT:3 * BT]
                c3 = cur[:, 3 * BT:4 * BT]
                s1 = out_sb[:, 0 * BT:1 * BT]
                s2 = out_sb[:, 1 * BT:2 * BT]
                s3 = out_sb[:, 2 * BT:3 * BT]

                m2 = lp.tile([P, BT], f32, tag="m2")
                nc.vector.scalar_tensor_tensor(m2[:], c0, 0.99, c1,
                                               A.mult, A.add)
                nc.vector.tensor_scalar(s1, m2[:], 1.0, None, A.is_gt)
                t2 = lp.tile([P, BT], f32, tag="t2")
                nc.vector.tensor_tensor(t2[:], c2, s1, A.subtract)
                m3 = lp.tile([P, BT], f32, tag="m3")
                nc.vector.scalar_tensor_tensor(m3[:], m2[:], 0.99, t2[:],
                                               A.mult, A.add)
                nc.vector.tensor_scalar(s2, m3[:], 1.0, None, A.is_gt)
                t3 = lp.tile([P, BT], f32, tag="t3")
                nc.vector.tensor_tensor(t3[:], c3, s2, A.subtract)
                m4 = lp.tile([P, BT], f32, tag="m4")
                nc.vector.scalar_tensor_tensor(m4[:], m3[:], 0.99, t3[:],
                                               A.mult, A.add)
                nc.vector.tensor_scalar(s3, m4[:], 1.0, None, A.is_gt)

                nc.sync.dma_start(out=out_d[j], in_=out_sb[:])

    nc.compile()
    return nc


def _blocked_w(Wt, KT, GJ):
    """[I, 3H] -> (j, p, g, k, m) blocked layout."""
    Wb = Wt.reshape(KT, P, 3, GJ, P).transpose(3, 1, 2, 0, 4)
    return np.ascontiguousarray(Wb)


def prep_weights(W_ih, b_ih, b_hh, KT, GJ, scheme=None):
    """Host-side packing of weights/biases (shared across cores)."""
    scheme = scheme or SCHEME
    threeH = 3 * GJ * P
    II = KT * P
    Wt = np.ascontiguousarray(W_ih[:threeH, :II].T)          # [I, 3H] fp32

    HH = GJ * P
    b_r = (b_ih[0:HH] + b_hh[0:HH]).astype(np.float32)
    b_z = (b_ih[HH:2 * HH] + b_hh[HH:2 * HH]).astype(np.float32)
    b_in = b_ih[2 * HH:3 * HH].astype(np.float32)
    b_hn = b_hh[2 * HH:3 * HH].astype(np.float32)
    asb = lambda b: np.ascontiguousarray(b.reshape(GJ, P).T)

    if scheme == "bf16x3":
        Wb = _blocked_w(Wt, KT, GJ)
        wh = Wb.astype(BF16)
        wl = (Wb - wh.astype(np.float32)).astype(BF16)
        return {"wh": wh, "wl": wl, "br": asb(b_r), "bz": asb(b_z),
                "bin": asb(b_in), "bhn": asb(b_hn)}

    Wb = _blocked_w(Wt, KT, GJ)                              # [GJ,P,3,KT,P]
    wh16 = Wb.astype(np.float16)
    wl = Wb - wh16.astype(np.float32)
    wh_scaled = (wh16.astype(np.float32) * SW_H).astype(np.float16)
    w8 = np.empty(Wb.shape[:4] + (2, P), dtype=FP8)
    w8[..., 0, :] = (wh16.astype(np.float32) * SW8_H).astype(FP8)
    w8[..., 1, :] = (wl * SW8_L).astype(FP8)
    return {"wh": wh_scaled, "w8": w8,
            "br": asb(-b_r * SCALE), "bz": asb(-b_z * SCALE),
            "bin": asb(b_in), "bhn": asb(b_hn)}


def prep_x(x_core, KT, BT, scheme=None):
    """x_core: [BL, I, T] fp32 -> per-core input dict (t-major cols)."""
    scheme = scheme or SCHEME
    II = KT * P
    xt = x_core[:, :II, :].transpose(1, 2, 0)          # [I, T, BL]
    xt = xt.reshape(KT, P, 4 * BT).transpose(1, 0, 2)  # [P, KT, TB]
    xt = np.ascontiguousarray(xt)
    if scheme == "bf16x3":
        xh = xt.astype(BF16)
        xl = (xt - xh.astype(np.float32)).astype(BF16)
        return {"xh": xh, "xl": xl}
    xh16 = xt.astype(np.float16)
    xl = xt - xh16.astype(np.float32)
    xh_scaled = (xh16.astype(np.float32) * SX_H).astype(np.float16)
    x8 = np.empty((P, KT, 2, 4 * BT), dtype=FP8)
    x8[:, :, 0, :] = (xl * SX8_L).astype(FP8)
    x8[:, :, 1, :] = xh16.astype(np.float32).astype(FP8)
    return {"xh": xh_scaled, "x8": x8}


def unpack_out(out, GJ, BT):
    """out: [GJ, P, 3*BT] fp32 -> spikes [BL, H', 4] with t=0 zeros."""
    HH = GJ * P
    arr = out.reshape(HH, 3, BT)                     # [h, t-1, b]
    res = np.zeros((BT, HH, 4), dtype=np.float32)
    res[:, :, 1:4] = arr.transpose(2, 0, 1)
    return res


def kernel(inputs, W_ih, b_ih, W_hh, b_hh):
    from concourse.bass_utils import run_bass_kernel_spmd

    # BT = batch rows per timestep per core (= local batch size BL)
    KT, GJ, BT = I // P, H // P, B // NCORES
    key = (KT, GJ, BT, SCHEME)
    if key not in _CACHE:
        _CACHE[key] = build_nc(KT, GJ, BT)
    nc = _CACHE[key]

    wmap = prep_weights(np.asarray(W_ih, dtype=np.float32),
                        np.asarray(b_ih, dtype=np.float32),
                        np.asarray(b_hh, dtype=np.float32), KT, GJ)

    x = np.asarray(inputs, dtype=np.float32)
    in_maps = []
    BL = B // NCORES
    for c in range(NCORES):
        m = dict(wmap)
        m.update(prep_x(x[c * BL:(c + 1) * BL], KT, BT))
        in_maps.append(m)

    res = run_bass_kernel_spmd(nc, in_maps, list(range(NCORES)), trace=TRACE)
    global LAST_EXEC_NS, LAST_RESULTS
    LAST_EXEC_NS = res.exec_time_ns
    LAST_RESULTS = res

    out = np.empty((B, H, T), dtype=np.float32)
    for c in range(NCORES):
        out[c * BL:(c + 1) * BL] = unpack_out(res.results[c]["out"], GJ, BT)
    return out


# revision 13
# speedup vs baseline: 1.7216x; 1.1967x over previous
"""Trainium2 Bass kernel for nn_GRUCell_21612275433682.

Math (from the reference):
  - h0 = 0, so the W_hh matmul is dead: only b_hh enters the gates.
  - y = x @ W_ih.T            (the single big GEMM, [B*T, I] @ [I, 3H])
  - r = (y_r + b_ih_r + b_hh_r > 0)
  - z = (y_z + b_ih_z + b_hh_z > 0)
  - n = (y_n + b_ih_n + r*b_hh_n > 0)
  - cur = (1-z)*n   in {0,1}
  - LIF over T=4 steps:  mem' = 0.99*mem + cur_t - spk_{t-1};  spk_t = (mem' > 1)
    spk_0 is identically 0 (mem1 = cur0 <= 1).

Strategy: pure data parallel over 8 cores (B sharded 256/core). Per core one
[3H=6144, TB=1024] x [I=2048] GEMM with W stationary ([I,3H] tiles) and X
moving. Output layout [3h partitions, (t-major t*BT+b) free] so biases are
per-partition scalars and the LIF is pure free-dim slicing.

Two GEMM precision schemes (SCHEME):
  - "bf16x3":  W,X split into bf16 hi/lo; 3 bf16 passes (hi*hi + hi*lo +
               lo*hi) accumulated in fp32 PSUM.  3 cycles/row.
  - "f16f8":   W,X split into fp16 hi/lo; 1 fp16 pass (hi*hi, products are
               exact in fp32) + both cross terms (hi*lo + lo*hi) packed into
               one fp8e4m3 DoubleRow pass.  Everything is pre-scaled by
               powers of two to a common 2^16 PSUM scale so all passes
               accumulate into one PSUM bank; the gate thresholds absorb the
               scale.  2 cycles/row (fp16 1.0 + fp8-DR ~0.56).
"""

import numpy as np
import ml_dtypes

BF16 = ml_dtypes.bfloat16
FP8 = ml_dtypes.float8_e4m3

# Full problem sizes (hardcoded per contract)
B, I, H, T = 2048, 2048, 2048, 4
NCORES = 8
P = 128

SCHEME = "f16f8"

# scheme f16f8 scale choices (powers of two; see product-scale table below)
#   main:  (wh * 2^8) @ (xh * 2^8)            -> y_main * 2^16
#   cross: fp8(wh*2^5) @ fp8(xl*2^11)         -> cross1 * 2^16
#          fp8(wl*2^16) @ fp8(xh)             -> cross2 * 2^16
SW_H, SX_H = 256.0, 256.0
SW8_H, SX8_L = 32.0, 2048.0
SW8_L, SX8_H = 65536.0, 1.0
SCALE = 65536.0

_CACHE = {}

# test-harness knobs (grading path leaves these alone)
TRACE = False
LAST_EXEC_NS = None
LAST_RESULTS = None


def _common_io(nc, mybir, KT, GJ, TB):
    f32 = mybir.dt.float32
    br_d = nc.dram_tensor("br", [P, GJ], f32, kind="ExternalInput")
    bz_d = nc.dram_tensor("bz", [P, GJ], f32, kind="ExternalInput")
    bin_d = nc.dram_tensor("bin", [P, GJ], f32, kind="ExternalInput")
    bhn_d = nc.dram_tensor("bhn", [P, GJ], f32, kind="ExternalInput")
    out_d = nc.dram_tensor("out", [GJ, P, 3 * (TB // 4)], f32,
                           kind="ExternalOutput")
    return br_d, bz_d, bin_d, bhn_d, out_d


def build_nc(KT, GJ, BT, scheme=None):
    """Build the per-core Bass program.

    KT: number of 128-wide K tiles (I = 128*KT)
    GJ: number of 128-row h-tile groups per gate (H = 128*GJ)
    BT: batch rows per timestep per core (TB = 4*BT total moving columns)
    """
    import concourse.mybir as mybir
    import concourse.tile as tile
    from concourse import bacc

    scheme = scheme or SCHEME
    TB = 4 * BT
    NT = TB // 512
    assert NT * 512 == TB

    f32 = mybir.dt.float32
    bf16 = mybir.dt.bfloat16
    f16 = mybir.dt.float16
    f8 = mybir.dt.float8e4
    A = mybir.AluOpType
    DR = mybir.MatmulPerfMode.DoubleRow

    nc = bacc.Bacc("TRN2", target_bir_lowering=False, debug=False,
                   num_devices=NCORES)

    if scheme == "bf16x3":
        xh_d = nc.dram_tensor("xh", [P, KT, TB], bf16, kind="ExternalInput")
        xl_d = nc.dram_tensor("xl", [P, KT, TB], bf16, kind="ExternalInput")
        wh_d = nc.dram_tensor("wh", [GJ, P, 3, KT, P], bf16,
                              kind="ExternalInput")
        wl_d = nc.dram_tensor("wl", [GJ, P, 3, KT, P], bf16,
                              kind="ExternalInput")
    else:
        xh_d = nc.dram_tensor("xh", [P, KT, TB], f16, kind="ExternalInput")
        x8_d = nc.dram_tensor("x8", [P, KT, 2, TB], f8, kind="ExternalInput")
        wh_d = nc.dram_tensor("wh", [GJ, P, 3, KT, P], f16,
                              kind="ExternalInput")
        w8_d = nc.dram_tensor("w8", [GJ, P, 2, KT, 2, P], f8,
                              kind="ExternalInput")
    br_d, bz_d, bin_d, bhn_d, out_d = _common_io(nc, mybir, KT, GJ, TB)

    with tile.TileContext(nc) as tc:
        with (
            tc.tile_pool(name="xp", bufs=1) as xp,
            tc.tile_pool(name="wp", bufs=2) as wp,
            tc.tile_pool(name="bp", bufs=1) as bp,
            tc.tile_pool(name="gp", bufs=2) as gp,
            tc.tile_pool(name="lp", bufs=2) as lp,
            tc.tile_pool(name="op", bufs=2) as op,
            tc.tile_pool(name="pp", bufs=7, space="PSUM") as pp,
        ):
            # X arrives on the ACT HWDGE ring in k-chunks so the first
            # matmuls (and the W loads on the sync ring) aren't stuck
            # behind one monolithic 8MB transfer.
            XC = 4 if KT % 4 == 0 else 1
            if scheme == "bf16x3":
                xh_sb = xp.tile([P, KT, TB], bf16, tag="xh")
                x2_sb = xp.tile([P, KT, TB], bf16, tag="x2")
                for c in range(0, KT, XC):
                    cs = slice(c, c + XC)
                    nc.scalar.dma_start(out=xh_sb[:, cs], in_=xh_d[:, cs])
                    nc.scalar.dma_start(out=x2_sb[:, cs], in_=xl_d[:, cs])
            else:
                xh_sb = xp.tile([P, KT, TB], f16, tag="xh")
                x2_sb = xp.tile([P, KT, 2, TB], f8, tag="x2")
                for c in range(0, KT, XC):
                    cs = slice(c, c + XC)
                    nc.scalar.dma_start(out=xh_sb[:, cs], in_=xh_d[:, cs])
                    nc.scalar.dma_start(out=x2_sb[:, cs], in_=x8_d[:, cs])

            # Warm the PE (HAM un-throttle needs ~3.4us of sustained matmul
            # activity) while the input DMAs land: dummy matmuls on a
            # memset tile into a spare PSUM bank.
            warm = bp.tile([P, 512], f16 if scheme != "bf16x3" else bf16,
                           tag="warm")
            nc.vector.memset(warm[:], 0)
            wps = pp.tile([P, 512], f32, tag="warmps", name="warmps",
                          bufs=1)
            for r_ in range(18):
                nc.tensor.matmul(wps[:], warm[:, 0:P], warm[:],
                                 start=(r_ == 0), stop=(r_ == 17),
                                 skip_group_check=True)

            br_sb = bp.tile([P, GJ], f32, tag="br")
            nc.gpsimd.dma_start(out=br_sb[:], in_=br_d[:])
            bz_sb = bp.tile([P, GJ], f32, tag="bz")
            nc.gpsimd.dma_start(out=bz_sb[:], in_=bz_d[:])
            bin_sb = bp.tile([P, GJ], f32, tag="bin")
            nc.gpsimd.dma_start(out=bin_sb[:], in_=bin_d[:])
            bhn_sb = bp.tile([P, GJ], f32, tag="bhn")
            nc.gpsimd.dma_start(out=bhn_sb[:], in_=bhn_d[:])

            for j in range(GJ):
                if scheme == "bf16x3":
                    wh_sb = wp.tile([P, 3, KT, P], bf16, tag="wh")
                    nc.sync.dma_start(out=wh_sb[:], in_=wh_d[j])
                    w2_sb = wp.tile([P, 3, KT, P], bf16, tag="w2")
                    nc.sync.dma_start(out=w2_sb[:], in_=wl_d[j])
                else:
                    wh_sb = wp.tile([P, 3, KT, P], f16, tag="wh")
                    nc.sync.dma_start(out=wh_sb[:], in_=wh_d[j])
                    w2_sb = wp.tile([P, 2, KT, 2, P], f8, tag="w2")
                    nc.sync.dma_start(out=w2_sb[:], in_=w8_d[j])

                cur = gp.tile([P, TB], f32, tag="cur")
                nsl = [slice(n * 512, (n + 1) * 512) for n in range(NT)]
                if scheme == "bf16x3":
                    psg = [[pp.tile([P, 512], f32, tag="ps",
                                     name=f"ps_{j}_{g}_{n}")
                            for n in range(NT)] for g in range(3)]
                    for g in range(3):
                        for n in range(NT):
                            pst = psg[g][n]
                            ns = nsl[n]
                            for k in range(KT):
                                xh_k = xh_sb[:, k, ns]
                                xl_k = x2_sb[:, k, ns]
                                nc.tensor.matmul(pst[:], wh_sb[:, g, k, :],
                                                 xh_k, start=(k == 0),
                                                 stop=False)
                                nc.tensor.matmul(pst[:], wh_sb[:, g, k, :],
                                                 xl_k, start=False, stop=False)
                                nc.tensor.matmul(pst[:], w2_sb[:, g, k, :],
                                                 xh_k, start=False,
                                                 stop=(k == KT - 1))
                else:
                    # Alternate 213ns fp16 MMs with 120ns fp8-DR MMs across
                    # all 3*NT PSUM banks so every 256-col DR weight-load
                    # hides under a preceding fp16 MM.
                    psg = [[pp.tile([P, 512], f32, tag="ps",
                                     name=f"ps_{j}_{g}_{n}")
                            for n in range(NT)] for g in range(3)]
                    # g=0 (r-gate) skips the fp8 correction: an r flip only
                    # matters when y_n lands inside the +-b_hn window
                    # (P ~ 0.8%), so fp16-main accuracy is plenty for r.
                    for k in range(KT):
                        for g in range(3):
                            w16 = wh_sb[:, g, k, :]
                            w8k = w2_sb[:, g - 1, k, :, :] if g else None
                            for n in range(NT):
                                nc.tensor.matmul(psg[g][n][:], w16,
                                                 xh_sb[:, k, nsl[n]],
                                                 start=(k == 0),
                                                 stop=(g == 0 and
                                                       k == KT - 1),
                                                 skip_group_check=True)
                                if g != 0:
                                    nc.tensor.matmul(psg[g][n][:], w8k,
                                                     x2_sb[:, k, :, nsl[n]],
                                                     perf_mode=DR,
                                                     start=False,
                                                     stop=(k == KT - 1),
                                                     skip_group_check=True)

                for n in range(NT):
                    ns = nsl[n]
                    ps = [psg[0][n], psg[1][n], psg[2][n]]
                    bj = lambda t: t[:, j:j + 1]
                    r = gp.tile([P, 512], f32, tag="r")
                    zb = gp.tile([P, 512], f32, tag="zb")
                    if scheme == "bf16x3":
                        # r = (y_r + b_r) > 0 ; zbar = (y_z + b_z) <= 0
                        nc.vector.tensor_scalar(r[:], ps[0][:], bj(br_sb), 0.0,
                                                A.add, A.is_gt)
                        nc.vector.tensor_scalar(zb[:], ps[1][:], bj(bz_sb),
                                                0.0, A.add, A.is_le)
                        # nn = y_n + b_in (ACT engine, matches ref rounding)
                        nn = gp.tile([P, 512], f32, tag="nn")
                        nc.scalar.activation(
                            nn[:], ps[2][:],
                            mybir.ActivationFunctionType.Identity,
                            bias=bj(bin_sb), scale=1.0)
                        # n2 = r*b_hn + nn ;  cur = (n2 > 0) * zbar
                        n2 = gp.tile([P, 512], f32, tag="n2")
                        nc.vector.scalar_tensor_tensor(n2[:], r[:], bj(bhn_sb),
                                                       nn[:], A.mult, A.add)
                        nc.vector.scalar_tensor_tensor(cur[:, ns], n2[:], 0.0,
                                                       zb[:], A.is_gt, A.mult)
                    else:
                        # psum holds y*2^16; br/bz arrive pre-scaled by
                        # -2^16 so the compare absorbs bias and scale.
                        nc.vector.tensor_scalar(r[:], ps[0][:], bj(br_sb),
                                                None, A.is_gt)
                        nc.vector.tensor_scalar(zb[:], ps[1][:], bj(bz_sb),
                                                None, A.is_le)
                        # rbn = r*b_hn + b_in ; n2 = y_n*2^-16 + rbn
                        rbn = gp.tile([P, 512], f32, tag="rbn")
                        nc.vector.tensor_scalar(rbn[:], r[:], bj(bhn_sb),
                                                bj(bin_sb), A.mult, A.add)
                        n2 = gp.tile([P, 512], f32, tag="n2")
                        nc.vector.scalar_tensor_tensor(n2[:], ps[2][:],
                                                       1.0 / SCALE, rbn[:],
                                                       A.mult, A.add)
                        nc.vector.scalar_tensor_tensor(cur[:, ns], n2[:], 0.0,
                                                       zb[:], A.is_gt, A.mult)

                # LIF over the 4 timesteps (t-major layout in cur)
                out_sb = op.tile([P, 3 * BT], f32, tag="out")
                c0 = cur[:, 0 * BT:1 * BT]
                c1 = cur[:, 1 * BT:2 * BT]
                c2 = cur[:, 2 * BT:3 * BT]
                c3 = cur[:, 3 * BT:4 * BT]
                s1 = out_sb[:, 0 * BT:1 * BT]
                s2 = out_sb[:, 1 * BT:2 * BT]
                s3 = out_sb[:, 2 * BT:3 * BT]

                m2 = lp.tile([P, BT], f32, tag="m2")
                nc.vector.scalar_tensor_tensor(m2[:], c0, 0.99, c1,
                                               A.mult, A.add)
                nc.vector.tensor_scalar(s1, m2[:], 1.0, None, A.is_gt)
                t2 = lp.tile([P, BT], f32, tag="t2")
                nc.vector.tensor_tensor(t2[:], c2, s1, A.subtract)
                m3 = lp.tile([P, BT], f32, tag="m3")
                nc.vector.scalar_tensor_tensor(m3[:], m2[:], 0.99, t2[:],
                                               A.mult, A.add)
                nc.vector.tensor_scalar(s2, m3[:], 1.0, None, A.is_gt)
                t3 = lp.tile([P, BT], f32, tag="t3")
                nc.vector.tensor_tensor(t3[:], c3, s2, A.subtract)
                m4 = lp.tile([P, BT], f32, tag="m4")
                nc.vector.scalar_tensor_tensor(m4[:], m3[:], 0.99, t3[:],
                                               A.mult, A.add)
                nc.vector.tensor_scalar(s3, m4[:], 1.0, None, A.is_gt)

                nc.gpsimd.dma_start(out=out_d[j], in_=out_sb[:])

    nc.compile()
    return nc


def _blocked_w(Wt, KT, GJ):
    """[I, 3H] -> (j, p, g, k, m) blocked layout."""
    Wb = Wt.reshape(KT, P, 3, GJ, P).transpose(3, 1, 2, 0, 4)
    return np.ascontiguousarray(Wb)


def prep_weights(W_ih, b_ih, b_hh, KT, GJ, scheme=None):
    """Host-side packing of weights/biases (shared across cores)."""
    scheme = scheme or SCHEME
    threeH = 3 * GJ * P
    II = KT * P
    Wt = np.ascontiguousarray(W_ih[:threeH, :II].T)          # [I, 3H] fp32

    HH = GJ * P
    b_r = (b_ih[0:HH] + b_hh[0:HH]).astype(np.float32)
    b_z = (b_ih[HH:2 * HH] + b_hh[HH:2 * HH]).astype(np.float32)
    b_in = b_ih[2 * HH:3 * HH].astype(np.float32)
    b_hn = b_hh[2 * HH:3 * HH].astype(np.float32)
    asb = lambda b: np.ascontiguousarray(b.reshape(GJ, P).T)

    if scheme == "bf16x3":
        Wb = _blocked_w(Wt, KT, GJ)
        wh = Wb.astype(BF16)
        wl = (Wb - wh.astype(np.float32)).astype(BF16)
        return {"wh": wh, "wl": wl, "br": asb(b_r), "bz": asb(b_z),
                "bin": asb(b_in), "bhn": asb(b_hn)}

    Wb = _blocked_w(Wt, KT, GJ)                              # [GJ,P,3,KT,P]
    wh16 = Wb.astype(np.float16)
    wl = Wb - wh16.astype(np.float32)
    wh_scaled = (wh16.astype(np.float32) * SW_H).astype(np.float16)
    # fp8 correction factors only for the z/n gates (g=1,2)
    w8 = np.empty((Wb.shape[0], P, 2, KT, 2, P), dtype=FP8)
    w8[:, :, :, :, 0, :] = (wh16[:, :, 1:3].astype(np.float32)
                            * SW8_H).astype(FP8)
    w8[:, :, :, :, 1, :] = (wl[:, :, 1:3] * SW8_L).astype(FP8)
    return {"wh": wh_scaled, "w8": w8,
            "br": asb(-b_r * SCALE), "bz": asb(-b_z * SCALE),
            "bin": asb(b_in), "bhn": asb(b_hn)}


def prep_x(x_core, KT, BT, scheme=None):
    """x_core: [BL, I, T] fp32 -> per-core input dict (t-major cols)."""
    scheme = scheme or SCHEME
    II = KT * P
    xt = x_core[:, :II, :].transpose(1, 2, 0)          # [I, T, BL]
    xt = xt.reshape(KT, P, 4 * BT).transpose(1, 0, 2)  # [P, KT, TB]
    xt = np.ascontiguousarray(xt)
    if scheme == "bf16x3":
        xh = xt.astype(BF16)
        xl = (xt - xh.astype(np.float32)).astype(BF16)
        return {"xh": xh, "xl": xl}
    xh16 = xt.astype(np.float16)
    xl = xt - xh16.astype(np.float32)
    xh_scaled = (xh16.astype(np.float32) * SX_H).astype(np.float16)
    x8 = np.empty((P, KT, 2, 4 * BT), dtype=FP8)
    x8[:, :, 0, :] = (xl * SX8_L).astype(FP8)
    x8[:, :, 1, :] = xh16.astype(np.float32).astype(FP8)
    return {"xh": xh_scaled, "x8": x8}


def unpack_out(out, GJ, BT):
    """out: [GJ, P, 3*BT] fp32 -> spikes [BL, H', 4] with t=0 zeros."""
    HH = GJ * P
    arr = out.reshape(HH, 3, BT)                     # [h, t-1, b]
    res = np.zeros((BT, HH, 4), dtype=np.float32)
    res[:, :, 1:4] = arr.transpose(2, 0, 1)
    return res


def kernel(inputs, W_ih, b_ih, W_hh, b_hh):
    from concourse.bass_utils import run_bass_kernel_spmd

    # BT = batch rows per timestep per core (= local batch size BL)
    KT, GJ, BT = I // P, H // P, B // NCORES
    key = (KT, GJ, BT, SCHEME)
    if key not in _CACHE:
        _CACHE[key] = build_nc(KT, GJ, BT)
    nc = _CACHE[key]

    wmap = prep_weights(np.asarray(W_ih, dtype=np.float32),
                        np.asarray(b_ih, dtype=np.float32),
                        np.asarray(b_hh, dtype=np.float32), KT, GJ)

    x = np.asarray(inputs, dtype=np.float32)
    in_maps = []
    BL = B // NCORES
    for c in range(NCORES):
        m = dict(wmap)
        m.update(prep_x(x[c * BL:(c + 1) * BL], KT, BT))
        in_maps.append(m)

    res = run_bass_kernel_spmd(nc, in_maps, list(range(NCORES)), trace=TRACE)
    global LAST_EXEC_NS, LAST_RESULTS
    LAST_EXEC_NS = res.exec_time_ns
    LAST_RESULTS = res

    out = np.empty((B, H, T), dtype=np.float32)
    for c in range(NCORES):
        out[c * BL:(c + 1) * BL] = unpack_out(res.results[c]["out"], GJ, BT)
    return out
